# revision 3
# baseline (speedup 1.0000x reference)
"""AttentionTSSA Trainium2 kernel — full-IO contract.

kernel(**inputs) takes the FULL inputs (x [8,512,128,128], qkv_w, temp,
out_w, out_b), shards data-parallel over batch across the 8 NeuronCores
(batch i -> core i), runs a Bass/Tile kernel per core, and returns the
full [8,512,128,128] float32 output.

Per-core computation (one batch, layout [d, n], d on partitions):
  w = qkv_w @ xb                        (fp32r matmuls, PE)
  norm2[d] = sum_n w^2                  (ACT Square + free-axis accum)
  logits[h,n] = sum_d w^2/norm2[d]      (PE, invnorm2-masked lhsT)
  P = exp(temp[h]*logits)               (ACT, per-partition scale)
  Pi = P / sum_h P                      (GPSIMD partition_all_reduce + DVE)
  S[h] = sum_n Pi                       (ACT copy + accum)
  dots[d] = sum_n Pi_bcast * w^2        (PE indicator-matmul + DVE TTR)
  attn = 1/(1+dots/(S+1e-8))
  o = -w * Pi_bcast * attn              (DVE scalar_tensor_tensor)
  y = out_w @ o + out_b                 (fp16 matmuls, ACT bias add)

w and Pi are held in SBUF as float16 between the three passes.
"""

import sys

sys.path.insert(0, "/opt/trn_rl_repo")

from contextlib import ExitStack

import numpy as np

import concourse.bass as bass
import concourse.tile as tile
from concourse import bacc, mybir
from concourse.bass_utils import run_bass_kernel_spmd

F32 = mybir.dt.float32
F32R = mybir.dt.float32r
F16 = mybir.dt.float16
AF = mybir.ActivationFunctionType
ALU = mybir.AluOpType

B = 8            # batch == number of cores
C = 512          # channels
H_IMG, W_IMG = 128, 128
N = H_IMG * W_IMG
HEADS = 8
HD = 64          # head dim
NT = 512         # tokens per tile
KD = 4           # 128-partition tiles of the channel dim
P = 128

_NC_CACHE = {}


def _build_nc(n_tokens=N, n_cores=B):
    NTILES = n_tokens // NT
    nc = bacc.Bacc("TRN2", target_bir_lowering=False, debug=False,
                   num_devices=n_cores)

    xb = nc.dram_tensor("xb", [C, n_tokens], F16, kind="ExternalInput").ap()
    qkvwT = nc.dram_tensor("qkvwT", [C, C], F16, kind="ExternalInput").ap()
    outwT = nc.dram_tensor("outwT", [C, C], F16, kind="ExternalInput").ap()
    ind = nc.dram_tensor("ind", [HEADS, C], F16, kind="ExternalInput").ap()
    temp = nc.dram_tensor("temp", [HEADS, 1], F32, kind="ExternalInput").ap()
    outb = nc.dram_tensor("outb", [C, 1], F32, kind="ExternalInput").ap()
    y = nc.dram_tensor("y", [C, n_tokens], F32, kind="ExternalOutput").ap()
    svec_dram = nc.dram_tensor("svec_scratch", [HEADS, 1], F32).ap()

    with tile.TileContext(nc) as tc, ExitStack() as top:
        const = top.enter_context(tc.tile_pool(name="const", bufs=1))
        persist = top.enter_context(tc.tile_pool(name="persist", bufs=1))

        # --- constants into SBUF -------------------------------------------
        qkvwT_sb = [const.tile([P, C], F16, name=f"qkvwT{k}") for k in range(KD)]
        outwT_sb = [const.tile([P, C], F16, name=f"outwT{k}") for k in range(KD)]
        for k in range(KD):
            nc.sync.dma_start(qkvwT_sb[k][:], qkvwT[k * P:(k + 1) * P, :])
            nc.sync.dma_start(outwT_sb[k][:], outwT[k * P:(k + 1) * P, :])
        ind_sb = const.tile([HEADS, C], F16, name="ind")
        nc.sync.dma_start(ind_sb[:], ind)
        temp_sb = const.tile([HEADS, 1], F32, name="temp")
        nc.sync.dma_start(temp_sb[:], temp)
        outb_sb = const.tile([P, KD], F32, name="outb")
        for k in range(KD):
            nc.sync.dma_start(outb_sb[:, k:k + 1], outb[k * P:(k + 1) * P, :])

        # --- persistent state ----------------------------------------------
        w_store = [persist.tile([P, n_tokens], F16, name=f"w{k}")
                   for k in range(KD)]
        norm2_part = persist.tile([P, KD * NTILES], F32, name="norm2p")
        dots_part = persist.tile([P, KD * NTILES], F32, name="dotsp")
        s_part = persist.tile([HEADS, NTILES], F32, name="sp")
        inv2 = persist.tile([P, KD], F32, name="inv2")
        lmat = persist.tile([P, KD * HEADS], F16, name="lmat")  # logits lhsT
        nattn = persist.tile([P, KD], F32, name="nattn")

        # =================== Phase 1: qkv matmul + norm2 ===================
        with ExitStack() as p1:
            xpool = p1.enter_context(tc.tile_pool(name="x", bufs=8))
            sqscr = p1.enter_context(tc.tile_pool(name="sqscr", bufs=2))
            wps = p1.enter_context(tc.tile_pool(name="wps", bufs=6, space="PSUM"))
            for t in range(NTILES):
                xs = []
                for kc in range(KD):
                    xt = xpool.tile([P, NT], F16, tag="x")
                    nc.sync.dma_start(
                        xt[:], xb[kc * P:(kc + 1) * P, t * NT:(t + 1) * NT])
                    xs.append(xt)
                for kd in range(KD):
                    wp = wps.tile([P, NT], F32, tag="wps")
                    for kc in range(KD):
                        nc.tensor.matmul(
                            wp[:],
                            lhsT=qkvwT_sb[kc][:, kd * P:(kd + 1) * P],
                            rhs=xs[kc][:],
                            start=(kc == 0), stop=(kc == KD - 1))
                    sq = sqscr.tile([P, NT], F16, tag="sqscr")
                    nc.scalar.activation(
                        sq[:], wp[:], AF.Square,
                        accum_out=norm2_part[:, kd * NTILES + t:kd * NTILES + t + 1])
                    nc.vector.tensor_copy(
                        w_store[kd][:, t * NT:(t + 1) * NT], wp[:])

        # --- finalize norm2 -> invnorm2 -> logits lhsT ---------------------
        nc.vector.memset(lmat[:], 0.0)
        for kd in range(KD):
            nc.vector.tensor_reduce(
                inv2[:, kd:kd + 1],
                norm2_part[:, kd * NTILES:(kd + 1) * NTILES],
                axis=mybir.AxisListType.X, op=ALU.add)
        nc.vector.reciprocal(inv2[:], inv2[:])
        for kd in range(KD):
            # head 2*kd lives on partitions 0..63, head 2*kd+1 on 64..127
            nc.vector.tensor_copy(
                lmat[0:HD, kd * HEADS + 2 * kd:kd * HEADS + 2 * kd + 1],
                inv2[0:HD, kd:kd + 1])
            nc.vector.tensor_copy(
                lmat[HD:P, kd * HEADS + 2 * kd + 1:kd * HEADS + 2 * kd + 2],
                inv2[HD:P, kd:kd + 1])

        # =================== Phase 2: softmax over heads + dots ============
        with ExitStack() as p2:
            pi_pool = p2.enter_context(tc.tile_pool(name="pi", bufs=1))
            pi_store = pi_pool.tile([HEADS, n_tokens], F16, name="pi")
            with ExitStack() as p2i:
                sqpool = p2i.enter_context(tc.tile_pool(name="sq", bufs=8))
                hpool = p2i.enter_context(tc.tile_pool(name="hp", bufs=3))
                scr = p2i.enter_context(tc.tile_pool(name="scr", bufs=2))
                lps = p2i.enter_context(
                    tc.tile_pool(name="lps", bufs=2, space="PSUM"))
                bps = p2i.enter_context(
                    tc.tile_pool(name="bps", bufs=4, space="PSUM"))
                for t in range(NTILES):
                    sqs = []
                    for kd in range(KD):
                        sq = sqpool.tile([P, NT], F16, tag="sq")
                        nc.scalar.activation(
                            sq[:], w_store[kd][:, t * NT:(t + 1) * NT], AF.Square)
                        sqs.append(sq)
                    lg = lps.tile([HEADS, NT], F32, tag="lps")
                    for kd in range(KD):
                        nc.tensor.matmul(
                            lg[:],
                            lhsT=lmat[:, kd * HEADS:(kd + 1) * HEADS],
                            rhs=sqs[kd][:],
                            start=(kd == 0), stop=(kd == KD - 1))
                    pt = hpool.tile([HEADS, NT], F32, tag="pt")
                    nc.scalar.activation(pt[:], lg[:], AF.Exp,
                                         scale=temp_sb[:, 0:1])
                    sm = hpool.tile([HEADS, NT], F32, tag="sm")
                    nc.gpsimd.partition_all_reduce(
                        sm[:], pt[:], channels=HEADS,
                        reduce_op=bass.bass_isa.ReduceOp.add)
                    ri = hpool.tile([HEADS, NT], F32, tag="ri")
                    nc.vector.reciprocal(ri[:], sm[:])
                    pi_t = pi_store[:, t * NT:(t + 1) * NT]
                    nc.vector.tensor_mul(pi_t, pt[:], ri[:])
                    pscr = scr.tile([HEADS, NT], F16, tag="pscr")
                    nc.scalar.activation(pscr[:], pi_t, AF.Copy,
                                         accum_out=s_part[:, t:t + 1])
                    for kd in range(KD):
                        pib = bps.tile([P, NT], F32, tag="bps")
                        nc.tensor.matmul(
                            pib[:],
                            lhsT=ind_sb[:, kd * P:(kd + 1) * P],
                            rhs=pi_t)
                        tscr = scr.tile([P, NT], F16, tag="tscr")
                        nc.vector.scalar_tensor_tensor(
                            out=tscr[:], in0=sqs[kd][:], scalar=1.0,
                            in1=pib[:], op0=ALU.mult, op1=ALU.mult,
                            accum_out=dots_part[:, kd * NTILES + t:
                                                kd * NTILES + t + 1])

                # --- finalize: S, dots, attn ------------------------------
                svec = hpool.tile([HEADS, 1], F32, tag="svec")
                nc.vector.tensor_reduce(svec[:], s_part[:],
                                        axis=mybir.AxisListType.X, op=ALU.add)
                nc.vector.tensor_scalar_add(svec[:], svec[:], 1e-8)
                nc.vector.reciprocal(svec[:], svec[:])
                # bounce 1/(S+eps) through DRAM to broadcast head values to
                # the per-d partition layout (engines can't shift/broadcast
                # partitions; DRAM-source DMA can).
                nc.sync.dma_start(svec_dram, svec[:])
                srb = hpool.tile([P, KD], F32, tag="srb")
                for kd in range(KD):
                    src = (svec_dram[2 * kd:2 * kd + 2, :]
                           .rearrange("h (r one) -> h r one", r=1)
                           .broadcast_to([2, HD, 1]))
                    nc.sync.dma_start(srb[:, kd:kd + 1], src)
                    dk = nattn[:, kd:kd + 1]
                    nc.vector.tensor_reduce(
                        dk, dots_part[:, kd * NTILES:(kd + 1) * NTILES],
                        axis=mybir.AxisListType.X, op=ALU.add)
                    # dots_n = dots * (1/(S+eps)); attn = 1/(1+dots_n)
                    nc.vector.tensor_scalar(
                        dk, dk, scalar1=srb[:, kd:kd + 1], scalar2=1.0,
                        op0=ALU.mult, op1=ALU.add)
                    nc.vector.reciprocal(dk, dk)
                    nc.vector.tensor_scalar_mul(dk, dk, -1.0)

            # =================== Phase 3: output + projection ==============
            with ExitStack() as p3:
                opool = p3.enter_context(tc.tile_pool(name="o", bufs=8))
                ypool = p3.enter_context(tc.tile_pool(name="y", bufs=8))
                b2ps = p3.enter_context(
                    tc.tile_pool(name="b2ps", bufs=4, space="PSUM"))
                ops = p3.enter_context(
                    tc.tile_pool(name="ops", bufs=4, space="PSUM"))
                for t in range(NTILES):
                    pi_t = pi_store[:, t * NT:(t + 1) * NT]
                    os_ = []
                    for kd in range(KD):
                        pib = b2ps.tile([P, NT], F32, tag="b2ps")
                        nc.tensor.matmul(
                            pib[:],
                            lhsT=ind_sb[:, kd * P:(kd + 1) * P],
                            rhs=pi_t)
                        ot = opool.tile([P, NT], F16, tag="o")
                        nc.vector.scalar_tensor_tensor(
                            out=ot[:],
                            in0=w_store[kd][:, t * NT:(t + 1) * NT],
                            scalar=nattn[:, kd:kd + 1],
                            in1=pib[:],
                            op0=ALU.mult, op1=ALU.mult)
                        os_.append(ot)
                    for kc in range(KD):
                        yp = ops.tile([P, NT], F32, tag="ops")
                        for kd in range(KD):
                            nc.tensor.matmul(
                                yp[:],
                                lhsT=outwT_sb[kd][:, kc * P:(kc + 1) * P],
                                rhs=os_[kd][:],
                                start=(kd == 0), stop=(kd == KD - 1))
                        yt = ypool.tile([P, NT], F32, tag="y")
                        nc.scalar.activation(yt[:], yp[:], AF.Identity,
                                             bias=outb_sb[:, kc:kc + 1],
                                             scale=1.0)
                        nc.sync.dma_start(
                            y[kc * P:(kc + 1) * P, t * NT:(t + 1) * NT], yt[:])

    nc.compile()
    return nc


def _host_inputs(x, qkv_w, temp, out_w, out_b):
    n_tokens = x.shape[2] * x.shape[3]
    qkvwT = np.ascontiguousarray(np.asarray(qkv_w).T).astype(np.float16)
    outwT = np.ascontiguousarray(np.asarray(out_w).T).astype(np.float16)
    ind = np.zeros((HEADS, C), np.float16)
    for d in range(C):
        ind[d // HD, d] = 1.0
    temp_a = np.asarray(temp, np.float32).reshape(HEADS, 1)
    outb_a = np.asarray(out_b, np.float32).reshape(C, 1)
    maps = []
    for i in range(x.shape[0]):
        maps.append({
            "xb": np.asarray(x[i], np.float32).reshape(C, n_tokens)
                .astype(np.float16),
            "qkvwT": qkvwT, "outwT": outwT, "ind": ind,
            "temp": temp_a, "outb": outb_a,
        })
    return maps


def kernel(x, qkv_w, temp, out_w, out_b):
    x = np.asarray(x)
    b, c, h, w = x.shape
    n_tokens = h * w
    key = (n_tokens, b)
    if key not in _NC_CACHE:
        _NC_CACHE[key] = _build_nc(n_tokens=n_tokens, n_cores=b)
    nc = _NC_CACHE[key]
    in_maps = _host_inputs(x, qkv_w, temp, out_w, out_b)
    res = run_bass_kernel_spmd(nc, in_maps, list(range(b)))
    out = np.stack([res.results[i]["y"].reshape(c, h, w) for i in range(b)])
    return out.astype(np.float32)
